# revision 41
# baseline (speedup 1.0000x reference)
"""BandSplitLinear Trainium2 kernel (bin-packed 128-row stripes, big DMAs).

Strategy (per core, batch-parallel over 8 cores):
  - No nonlinearity between the two per-band linears -> fold w_pre @ w_post
    into one (w_k*C x w_k*C) matrix per band on the host. Biases are additive
    constants per (c, f) -> applied host-side (zero in practice).
  - In packed feature order r = f*C + c the folded weight matrix is
    block-diagonal with 45 square blocks. Bin-pack the 45 bands into G=33
    bins of total width <= 128 (optimal: ceil(4100/128)); the host lays x
    out as fp16 stripes (pad rows zero-weighted), so the whole model is G
    uniform 128x128 matmuls with the contraction axis already on partitions:
    no transposes, gathers or scatters on device.
  - The kernel is HBM-bound (~420 GB/s/core measured): fp16 in/out halves
    traffic vs fp32; fp32 accumulate in PSUM. Per-dma_start descriptor-gen
    costs ~630ns (HWDGE, globally serialized) so transfers are batched into
    ~1MB multi-stripe pieces. HBM uses a partition-major layout [128, G*T]
    so each piece is one contiguous 2D slice with multi-KB descriptors.
  - Loads alternate the two fast HWDGE rings (sync/scalar); stores rotate
    all three rings; the slower gpsimd SWDGE ring carries the weights and
    a third of the stores. psum->sbuf cast copies alternate vector/scalar.
"""

import contextlib

import numpy as np

import concourse.bass as bass
import concourse.tile as tile
from concourse import bacc, mybir
from concourse.bass_utils import run_bass_kernel_spmd


# ---- problem constants (hardcoded per spec) ----
B, C, T, F = 8, 4, 1000, 1025
N_CORES = 8
P = 128
RTOT = F * C  # 4100 dense packed rows (r = f*C + c)
TC = 500  # matmul free-dim chunk (<= 512 fp32 PSUM bank)
NTC = T // TC  # 2
SPLIT = 4  # stripes per DMA piece

_F32 = mybir.dt.float32
_F16 = mybir.dt.float16


def _build_bands():
    f, interval = 0, 4
    groups = []
    while f < F:
        end = min(f + interval, F)
        groups.append((f, end))
        f = end
        if interval < 32:
            interval += 1
    return groups  # 45 disjoint (start, end) covering [0, F)


def _make_bins():
    """First-fit-decreasing bin packing of band widths into 128-row bins."""
    bands = _build_bands()
    sizes = [(e - s) * C for s, e in bands]
    order = sorted(range(len(sizes)), key=lambda k: -sizes[k])
    bins, fill = [], []
    for k in order:
        for i in range(len(bins)):
            if fill[i] + sizes[k] <= P:
                bins[i].append(k)
                fill[i] += sizes[k]
                break
        else:
            bins.append([k])
            fill.append(sizes[k])
    for b in bins:
        b.sort()
    bins.sort(key=lambda b: b[0])
    return bands, sizes, bins


def _layout():
    """perm: padded row -> dense row (or -1); pos: dense row -> padded row."""
    bands, sizes, bins = _make_bins()
    G = len(bins)
    perm = np.zeros(G * P, dtype=np.int64)  # pad rows point at 0 (zero weight)
    pos = np.empty(RTOT, dtype=np.int64)
    for g, bn in enumerate(bins):
        lb = 0
        for k in bn:
            si = sizes[k]
            dense0 = bands[k][0] * C
            perm[g * P + lb : g * P + lb + si] = np.arange(dense0, dense0 + si)
            pos[dense0 : dense0 + si] = np.arange(g * P + lb, g * P + lb + si)
            lb += si
    return bands, sizes, bins, G, perm, pos


def _bin_meta(sizes, bins):
    """Per-bin (width, tight column offset into the dense wall)."""
    wgs = [sum(sizes[k] for k in b) for b in bins]
    offs = np.concatenate([[0], np.cumsum(wgs)])
    return wgs, offs  # offs[-1] == RTOT


def _build_wall(w_pre, w_post, sizes, bins, bands, G):
    """Host: fold per-band linears, place blocks diagonally inside each bin."""
    wc = np.einsum("kio,kod->kid", w_pre.astype(np.float64), w_post.astype(np.float64))
    wall = np.zeros((P, G * P), dtype=np.float16)
    for g, bn in enumerate(bins):
        lb = 0
        for k in bn:
            si = sizes[k]
            wall[lb : lb + si, g * P + lb : g * P + lb + si] = wc[k][:si, :si].astype(
                np.float16
            )
            lb += si
    return wall


def _bias_field(bands, b_pre, w_post, b_post):
    """bias[c, f]: the constant added to out[., c, ., f]."""
    bc = (
        np.einsum("ko,kod->kd", b_pre.astype(np.float64), w_post.astype(np.float64))
        + b_post.astype(np.float64)
    )
    field = np.zeros((C, F), dtype=np.float64)
    for k, (start, end) in enumerate(bands):
        for c in range(C):
            field[c, start:end] = bc[k, (np.arange(end - start)) * C + c]
    return field.astype(np.float32)


def _pieces(G):
    out = []
    s = 0
    while s < G:
        out.append((s, min(s + SPLIT, G)))
        s += SPLIT
    return out  # [(g0, g1)) stripe ranges per DMA piece


def _build_nc(G):
    nc = bacc.Bacc("TRN2", target_bir_lowering=False, debug=False)
    # partition-major HBM layout: row p holds [g0: t0..T-1, g1: ...] so a
    # multi-stripe piece is one contiguous 2D slice with n*2KB descriptors
    xt = nc.dram_tensor("xt", [P, G * T], _F16, kind="ExternalInput")
    wall = nc.dram_tensor("wall", [P, G * P], _F16, kind="ExternalInput")
    ys = nc.dram_tensor("ys", [P, G * T], _F16, kind="ExternalOutput")
    pieces = _pieces(G)

    with tile.TileContext(nc) as tc:
        with contextlib.ExitStack() as ctx:
            const_pool = ctx.enter_context(tc.tile_pool(name="const", bufs=1))
            x_pool = ctx.enter_context(tc.tile_pool(name="xp", bufs=len(pieces)))
            y_pool = ctx.enter_context(tc.tile_pool(name="yp", bufs=len(pieces)))
            ps_pool = ctx.enter_context(tc.tile_pool(name="ps", bufs=8, space="PSUM"))

            # Queue discipline (measured): HWDGE rings (sync/scalar) sustain
            # ~210 GB/s each and together hit the ~430 GB/s core ceiling; the
            # gpsimd SWDGE ring is slower (~250-300) so it only carries the
            # wall and every third store.
            queues = [nc.sync, nc.scalar, nc.gpsimd]
            # wall at the head of the scalar ring: it transfers in parallel
            # with sync's first x piece, so the first matmul starts ~12us
            # instead of ~21us (on the gpsimd ring it was round-robin starved
            # behind the loads, stalling the whole matmul+copy chain)
            wall_sb = const_pool.tile([P, G * P], _F16)
            nc.scalar.dma_start(wall_sb[:], wall.ap())

            xp, yp = [], []
            for pi, (g0, g1) in enumerate(pieces):
                n = g1 - g0
                t_ = x_pool.tile([P, n * T], _F16, name="xp")
                queues[pi % 2].dma_start(t_[:], xt.ap()[:, g0 * T : g1 * T])
                xp.append(t_)
                yp.append(y_pool.tile([P, n * T], _F16, name="yp"))

            copy_engines = [
                lambda d, s_: nc.vector.tensor_copy(d, s_),
                lambda d, s_: nc.scalar.copy(d, s_),
            ]
            nco = 0
            for s, (g0, g1) in enumerate(pieces):
                for g in range(g0, g1):
                    o = g - g0
                    for ci in range(NTC):
                        ps = ps_pool.tile([P, TC], _F32, name="ps")
                        nc.tensor.matmul(
                            ps[:],
                            lhsT=wall_sb[:, g * P : (g + 1) * P],
                            rhs=xp[s][:, o * T + ci * TC : o * T + (ci + 1) * TC],
                            start=True,
                            stop=True,
                        )
                        copy_engines[nco % 2](
                            yp[s][:, o * T + ci * TC : o * T + (ci + 1) * TC], ps[:]
                        )
                        nco += 1
                # never on scalar: a backpressured store issue there would
                # stall the copies queued behind it and wedge the chain
                sq = nc.sync if s % 2 == 0 else nc.gpsimd
                sq.dma_start(ys.ap()[:, g0 * T : g1 * T], yp[s][:])
    nc.compile()
    return nc


_CACHE = {}


def _prepare(x, w_pre, w_post):
    """Returns (nc, in_maps) ready for run_bass_kernel_spmd."""
    bands, sizes, bins, G, perm, _pos = _layout()
    wall = _build_wall(w_pre, w_post, sizes, bins, bands, G)
    if "nc" not in _CACHE:
        _CACHE["nc"] = _build_nc(G)
    xt16 = np.ascontiguousarray(
        x.transpose(0, 3, 1, 2).reshape(B, RTOT, T), dtype=np.float16
    )
    # partition-major: device row p = [stripe g0 t-range, stripe g1, ...]
    perm_pm = perm.reshape(G, P).T.ravel()
    xt_pad = xt16[:, perm_pm, :].reshape(B, P, G * T)
    in_maps = [{"xt": xt_pad[b], "wall": wall} for b in range(N_CORES)]
    return _CACHE["nc"], in_maps


def kernel(x, w_pre, b_pre, w_post, b_post):
    x = np.asarray(x, dtype=np.float32)
    w_pre = np.asarray(w_pre, dtype=np.float32)
    b_pre = np.asarray(b_pre, dtype=np.float32)
    w_post = np.asarray(w_post, dtype=np.float32)
    b_post = np.asarray(b_post, dtype=np.float32)

    bands, _sizes, _bins, G, _perm, pos = _layout()
    nc, in_maps = _prepare(x, w_pre, w_post)
    res = run_bass_kernel_spmd(nc, in_maps, core_ids=list(range(N_CORES)))
    ys_all = np.stack([res.results[b]["ys"] for b in range(N_CORES)])

    # [B, P, G*T] -> dense rows -> [B, C, T, F]
    pos_pm = (pos % P) * G + pos // P  # dense row -> partition-major row
    yt = ys_all.reshape(B, P * G, T)[:, pos_pm, :]
    out = (
        yt.reshape(B, F, C, T).transpose(0, 2, 3, 1).astype(np.float32)
    )

    if np.any(b_pre) or np.any(b_post):
        field = _bias_field(bands, b_pre, w_post, b_post)
        out = out + field[None, :, None, :]
    return out


# revision 43
# speedup vs baseline: 1.1561x; 1.1561x over previous
"""BandSplitLinear Trainium2 kernel (bin-packed 128-row stripes, big DMAs).

Strategy (per core, batch-parallel over 8 cores):
  - No nonlinearity between the two per-band linears -> fold w_pre @ w_post
    into one (w_k*C x w_k*C) matrix per band on the host. Biases are additive
    constants per (c, f) -> applied host-side (zero in practice).
  - In packed feature order r = f*C + c the folded weight matrix is
    block-diagonal with 45 square blocks. Bin-pack the 45 bands into G=33
    bins of total width <= 128 (optimal: ceil(4100/128)); the host lays x
    out as fp16 stripes (pad rows zero-weighted), so the whole model is G
    uniform 128x128 matmuls with the contraction axis already on partitions:
    no transposes, gathers or scatters on device.
  - The kernel is HBM-bound (~420 GB/s/core measured): fp16 in/out halves
    traffic vs fp32; fp32 accumulate in PSUM. Per-dma_start descriptor-gen
    costs ~630ns (HWDGE, globally serialized) so transfers are batched into
    ~1MB multi-stripe pieces. HBM uses a partition-major layout [128, G*T]
    so each piece is one contiguous 2D slice with multi-KB descriptors.
  - Loads alternate the two fast HWDGE rings (sync/scalar); stores rotate
    all three rings; the slower gpsimd SWDGE ring carries the weights and
    a third of the stores. psum->sbuf cast copies alternate vector/scalar.
"""

import contextlib

import numpy as np

import concourse.bass as bass
import concourse.tile as tile
from concourse import bacc, mybir
from concourse.bass_utils import run_bass_kernel_spmd


# ---- problem constants (hardcoded per spec) ----
B, C, T, F = 8, 4, 1000, 1025
N_CORES = 8
P = 128
RTOT = F * C  # 4100 dense packed rows (r = f*C + c)
TC = 500  # matmul free-dim chunk (<= 512 fp32 PSUM bank)
NTC = T // TC  # 2
SPLIT = 4  # stripes per DMA piece

_F32 = mybir.dt.float32
_F16 = mybir.dt.float16


def _build_bands():
    f, interval = 0, 4
    groups = []
    while f < F:
        end = min(f + interval, F)
        groups.append((f, end))
        f = end
        if interval < 32:
            interval += 1
    return groups  # 45 disjoint (start, end) covering [0, F)


def _make_bins():
    """First-fit-decreasing bin packing of band widths into 128-row bins."""
    bands = _build_bands()
    sizes = [(e - s) * C for s, e in bands]
    order = sorted(range(len(sizes)), key=lambda k: -sizes[k])
    bins, fill = [], []
    for k in order:
        for i in range(len(bins)):
            if fill[i] + sizes[k] <= P:
                bins[i].append(k)
                fill[i] += sizes[k]
                break
        else:
            bins.append([k])
            fill.append(sizes[k])
    for b in bins:
        b.sort()
    bins.sort(key=lambda b: b[0])
    return bands, sizes, bins


def _layout():
    """perm: padded row -> dense row (or -1); pos: dense row -> padded row."""
    bands, sizes, bins = _make_bins()
    G = len(bins)
    perm = np.zeros(G * P, dtype=np.int64)  # pad rows point at 0 (zero weight)
    pos = np.empty(RTOT, dtype=np.int64)
    for g, bn in enumerate(bins):
        lb = 0
        for k in bn:
            si = sizes[k]
            dense0 = bands[k][0] * C
            perm[g * P + lb : g * P + lb + si] = np.arange(dense0, dense0 + si)
            pos[dense0 : dense0 + si] = np.arange(g * P + lb, g * P + lb + si)
            lb += si
    return bands, sizes, bins, G, perm, pos


def _bin_meta(sizes, bins):
    """Per-bin (width, tight column offset into the dense wall)."""
    wgs = [sum(sizes[k] for k in b) for b in bins]
    offs = np.concatenate([[0], np.cumsum(wgs)])
    return wgs, offs  # offs[-1] == RTOT


def _build_wall(w_pre, w_post, sizes, bins, bands, G):
    """Host: fold per-band linears, place blocks diagonally inside each bin."""
    wc = np.einsum("kio,kod->kid", w_pre.astype(np.float64), w_post.astype(np.float64))
    wall = np.zeros((P, G * P), dtype=np.float16)
    for g, bn in enumerate(bins):
        lb = 0
        for k in bn:
            si = sizes[k]
            wall[lb : lb + si, g * P + lb : g * P + lb + si] = wc[k][:si, :si].astype(
                np.float16
            )
            lb += si
    return wall


def _bias_field(bands, b_pre, w_post, b_post):
    """bias[c, f]: the constant added to out[., c, ., f]."""
    bc = (
        np.einsum("ko,kod->kd", b_pre.astype(np.float64), w_post.astype(np.float64))
        + b_post.astype(np.float64)
    )
    field = np.zeros((C, F), dtype=np.float64)
    for k, (start, end) in enumerate(bands):
        for c in range(C):
            field[c, start:end] = bc[k, (np.arange(end - start)) * C + c]
    return field.astype(np.float32)


def _pieces(G):
    out = []
    s = 0
    while s < G:
        out.append((s, min(s + SPLIT, G)))
        s += SPLIT
    return out  # [(g0, g1)) stripe ranges per DMA piece


def _build_nc(G):
    nc = bacc.Bacc("TRN2", target_bir_lowering=False, debug=False)
    # partition-major HBM layout: row p holds [g0: t0..T-1, g1: ...] so a
    # multi-stripe piece is one contiguous 2D slice with n*2KB descriptors
    xt = nc.dram_tensor("xt", [P, G * T], _F16, kind="ExternalInput")
    wall = nc.dram_tensor("wall", [P, G * P], _F16, kind="ExternalInput")
    ys = nc.dram_tensor("ys", [P, G * T], _F16, kind="ExternalOutput")
    pieces = _pieces(G)

    with tile.TileContext(nc) as tc:
        with contextlib.ExitStack() as ctx:
            const_pool = ctx.enter_context(tc.tile_pool(name="const", bufs=1))
            x_pool = ctx.enter_context(tc.tile_pool(name="xp", bufs=len(pieces)))
            y_pool = ctx.enter_context(tc.tile_pool(name="yp", bufs=len(pieces)))
            ps_pool = ctx.enter_context(tc.tile_pool(name="ps", bufs=8, space="PSUM"))

            # Queue discipline (measured): HWDGE rings (sync/scalar) sustain
            # ~210 GB/s each and together hit the ~430 GB/s core ceiling; the
            # gpsimd SWDGE ring is slower (~250-300) so it only carries the
            # wall and every third store.
            queues = [nc.sync, nc.scalar, nc.gpsimd]
            wall_sb = const_pool.tile([P, G * P], _F16)
            nc.gpsimd.dma_start(wall_sb[:], wall.ap())

            xp, yp = [], []
            for pi, (g0, g1) in enumerate(pieces):
                n = g1 - g0
                t_ = x_pool.tile([P, n * T], _F16, name="xp")
                queues[pi % 2].dma_start(t_[:], xt.ap()[:, g0 * T : g1 * T])
                xp.append(t_)
                yp.append(y_pool.tile([P, n * T], _F16, name="yp"))

            copy_engines = [
                lambda d, s_: nc.vector.tensor_copy(d, s_),
                lambda d, s_: nc.scalar.copy(d, s_),
            ]
            nco = 0
            for s, (g0, g1) in enumerate(pieces):
                for g in range(g0, g1):
                    o = g - g0
                    for ci in range(NTC):
                        ps = ps_pool.tile([P, TC], _F32, name="ps")
                        nc.tensor.matmul(
                            ps[:],
                            lhsT=wall_sb[:, g * P : (g + 1) * P],
                            rhs=xp[s][:, o * T + ci * TC : o * T + (ci + 1) * TC],
                            start=True,
                            stop=True,
                        )
                        copy_engines[nco % 2](
                            yp[s][:, o * T + ci * TC : o * T + (ci + 1) * TC], ps[:]
                        )
                        nco += 1
                queues[s % 3].dma_start(ys.ap()[:, g0 * T : g1 * T], yp[s][:])
    nc.compile()
    return nc


_CACHE = {}


def _prepare(x, w_pre, w_post):
    """Returns (nc, in_maps) ready for run_bass_kernel_spmd."""
    bands, sizes, bins, G, perm, _pos = _layout()
    wall = _build_wall(w_pre, w_post, sizes, bins, bands, G)
    if "nc" not in _CACHE:
        _CACHE["nc"] = _build_nc(G)
    xt16 = np.ascontiguousarray(
        x.transpose(0, 3, 1, 2).reshape(B, RTOT, T), dtype=np.float16
    )
    # partition-major: device row p = [stripe g0 t-range, stripe g1, ...]
    perm_pm = perm.reshape(G, P).T.ravel()
    xt_pad = xt16[:, perm_pm, :].reshape(B, P, G * T)
    in_maps = [{"xt": xt_pad[b], "wall": wall} for b in range(N_CORES)]
    return _CACHE["nc"], in_maps


def kernel(x, w_pre, b_pre, w_post, b_post):
    x = np.asarray(x, dtype=np.float32)
    w_pre = np.asarray(w_pre, dtype=np.float32)
    b_pre = np.asarray(b_pre, dtype=np.float32)
    w_post = np.asarray(w_post, dtype=np.float32)
    b_post = np.asarray(b_post, dtype=np.float32)

    bands, _sizes, _bins, G, _perm, pos = _layout()
    nc, in_maps = _prepare(x, w_pre, w_post)
    res = run_bass_kernel_spmd(nc, in_maps, core_ids=list(range(N_CORES)))
    ys_all = np.stack([res.results[b]["ys"] for b in range(N_CORES)])

    # [B, P, G*T] -> dense rows -> [B, C, T, F]
    pos_pm = (pos % P) * G + pos // P  # dense row -> partition-major row
    yt = ys_all.reshape(B, P * G, T)[:, pos_pm, :]
    out = (
        yt.reshape(B, F, C, T).transpose(0, 2, 3, 1).astype(np.float32)
    )

    if np.any(b_pre) or np.any(b_post):
        field = _bias_field(bands, b_pre, w_post, b_post)
        out = out + field[None, :, None, :]
    return out
